# revision 12
# baseline (speedup 1.0000x reference)
"""MinGRU cell on 8 Trainium2 NeuronCores.

Math: the reference computes, per (batch b, hidden channel j), the linear
recurrence (written there in log-space for stability)

    h_t = c_t * h_{t-1} + v_t,      h_0 = g(h0)
    c_t = 1 - sigmoid(kz_t) = sigmoid(-kz_t)
    v_t = sigmoid(kz_t) * g(kh_t)
    kz = x @ Wz^T + bz,  kh = x @ Wh^T + bh
    g(u) = u + 0.5 if u >= 0 else sigmoid(u) = max(sigmoid(u), u + 0.5)
    (the max identity is exact: u + 0.5 - sigmoid(u) is 0 at u=0 and increasing)

All quantities are positive and O(1) (h_t is a convex combination), so the
linear-space recurrence is numerically fine in fp32 (~1.3e-3 max rel err vs
the log-space reference).

Sharding: data-parallel over batch, one batch row per core (B == 8 cores).
Weights replicated. Each core computes kz/kh with the tensor engine in a
[h-partition, s-free] layout; the recurrence runs as tensor_tensor_scan
along the free (s) axis.

v2 pipeline (vs v1): two 512-col s-blocks are accumulated into one
[128, 1024] PSUM tile (2 banks) per gate, so every post-matmul op covers
1024 columns in a single instruction — half the per-instruction overhead
on the Activation engine (exec-queue depth 0 exposes each instruction's
latency) and half the scan count on DVE. The hardware scan runs ~2.4x
slower than the cost model (2.4 ns/elem) and only exists on DVE, so DVE
load is what the post-matmul engine split minimizes (scheme E, default):
  Act : z = sig(kz+bz), a = sig(kh+bh), m = kh+bh+0.5   (one act table;
        Identity and Sigmoid share it, so no table reloads)
  DVE : c = 1 - z [tensor_scalar], tl = max(a, m), ot = scan(c, v)
  Pool: v = z * tl  [gpsimd tensor_tensor, keeps DVE free for scans]
(1 - z instead of sig(-kz-bz) is safe: the absolute error is < 1 ulp of
1.0 and c only weights h_prev, so the output error stays ~1e-7.)
Scan chains across s via a 10-deep ring of output tiles (init reads the
previous tile's last column directly). PE accumulation groups are kept
contiguous per 512-col PSUM slice — interleaving two open groups stalls
the PE array (measured 2.4x matmul slowdown).

Host-side layout only (no math): x is fed pre-transposed (D, S) per batch so
the contraction dim lands on partitions; output comes back (H, S) and is
transposed on the host.
"""

import numpy as np

import concourse.bass as bass
import concourse.mybir as mybir
import concourse.tile as tile
from concourse import bacc
from concourse.bass_utils import run_bass_kernel_spmd

B, S, D, H = 8, 4096, 1024, 1024
N_CORES = 8
P = 128              # partitions
SB = 512             # columns per matmul / PSUM bank
SF = 2               # s-blocks fused per PSUM tile
SBF = SB * SF        # 1024 columns per fused tile
NSB = S // SB        # 8
NSBF = S // SBF      # 4
DT = D // P          # 8 contraction tiles
HT = H // P          # 8 hidden tiles

F32 = mybir.dt.float32
# Matmul operand dtype: float32 (exact, 4 cyc/row) or float32r (1 cyc/row).
MM_DT = mybir.dt.float32r
# Post-matmul engine split (see _build_program docstring).
SCHEME = "E"

_CACHE = {}


def _build_program(ablate=(), repeat=1, mm_dt=None, bufs=None, vt_pool=True,
                   scheme=None):
    """ablate: subset of {'mm','act','dve','scan','outdma','xdma'} to stub out.
    repeat: unroll the whole body N times (timing only; results identical).
    vt_pool: compute v = z*tl on gpsimd (Pool) instead of DVE.
    scheme: 'A' = Act{z,c,a}, DVE{tl-stt, scan};
            'E' = Act{z,a,m}, DVE{c-ts, tl-max, scan} (less DVE time)."""
    if mm_dt is None:
        mm_dt = MM_DT
    if scheme is None:
        scheme = SCHEME
    bufs = {**{"xin": 3, "psz": 2, "psh": 2, "inter": 2, "outp": 10},
            **(bufs or {})}
    nc = bacc.Bacc(trn_type="TRN2")

    xT = nc.dram_tensor("xt", [D, S], mm_dt, kind="ExternalInput")
    wzT = nc.dram_tensor("wzt", [D, H], mm_dt, kind="ExternalInput")
    whT = nc.dram_tensor("wht", [D, H], mm_dt, kind="ExternalInput")
    bzg = nc.dram_tensor("bzg", [P, HT], F32, kind="ExternalInput")
    bhg = nc.dram_tensor("bhg", [P, HT], F32, kind="ExternalInput")
    h0g = nc.dram_tensor("h0g", [P, HT], F32, kind="ExternalInput")
    hT = nc.dram_tensor("ht", [H, S], F32, kind="ExternalOutput")

    AF = mybir.ActivationFunctionType
    OP = mybir.AluOpType

    with tile.TileContext(nc) as tc:
        with (
            tc.tile_pool(name="wpool", bufs=1) as wpool,
            tc.tile_pool(name="bias", bufs=1) as bias,
            tc.tile_pool(name="xin", bufs=bufs["xin"]) as xin,
            tc.tile_pool(name="psz", bufs=bufs["psz"], space="PSUM") as psz,
            tc.tile_pool(name="psh", bufs=bufs["psh"], space="PSUM") as psh,
            tc.tile_pool(name="inter", bufs=bufs["inter"]) as inter,
            tc.tile_pool(name="outp", bufs=bufs["outp"]) as outp,
        ):
            # Weights, laid out [p(d-in-tile), d-tile, h]; chunked per d-tile
            # so the first matmuls can start before all weights land.
            wz_sb = wpool.tile([P, DT, H], mm_dt, tag="wz")
            wh_sb = wpool.tile([P, DT, H], mm_dt, tag="wh")
            wzT_v = wzT.ap().rearrange("(dt p) h -> p dt h", p=P)
            whT_v = whT.ap().rearrange("(dt p) h -> p dt h", p=P)
            for di in range(DT):
                nc.sync.dma_start(
                    out=wz_sb[:, di:di + 1, :], in_=wzT_v[:, di:di + 1, :]
                )
                nc.sync.dma_start(
                    out=wh_sb[:, di:di + 1, :], in_=whT_v[:, di:di + 1, :]
                )

            # Bias / initial-state columns, [p(h-in-tile), h-tile]
            bz_sb = bias.tile([P, HT], F32, tag="bz")
            nc.sync.dma_start(out=bz_sb, in_=bzg.ap())
            bh_sb = bias.tile([P, HT], F32, tag="bh")
            nc.sync.dma_start(out=bh_sb, in_=bhg.ap())
            h0_sb = bias.tile([P, HT], F32, tag="h0")
            nc.sync.dma_start(out=h0_sb, in_=h0g.ap())

            nbz_sb = bias.tile([P, HT], F32, tag="nbz")
            nc.vector.tensor_scalar_mul(nbz_sb[:], bz_sb[:], -1.0)
            bhh_sb = bias.tile([P, HT], F32, tag="bhh")  # bh + 0.5
            nc.vector.tensor_scalar_add(bhh_sb[:], bh_sb[:], 0.5)

            # g0 = max(sigmoid(h0), h0 + 0.5)
            g0_s = bias.tile([P, HT], F32, tag="g0s")
            nc.scalar.activation(g0_s[:], h0_sb[:], AF.Sigmoid)
            g0_t = bias.tile([P, HT], F32, tag="g0t")
            nc.vector.tensor_scalar_add(g0_t[:], h0_sb[:], 0.5)
            g0 = bias.tile([P, HT], F32, tag="g0")
            nc.vector.tensor_max(g0[:], g0_s[:], g0_t[:])

            xT_v = xT.ap().rearrange("(dt p) s -> p dt s", p=P)
            hT_v = hT.ap().rearrange("(ht p) s -> p ht s", p=P)

            vt_eng = nc.gpsimd if vt_pool else nc.vector

            for _rep in range(repeat):
              prev_out = [None] * HT
              x_t = [None, None]
              for sbp in range(NSBF):
                for j in range(SF):
                    x_t[j] = xin.tile([P, DT, SB], mm_dt, tag="x", name="xt_sb")
                    if "xdma" not in ablate:
                        sb = sbp * SF + j
                        nc.sync.dma_start(
                            out=x_t[j], in_=xT_v[:, :, sb * SB:(sb + 1) * SB]
                        )

                for hi in range(HT):
                    kz = psz.tile([P, SBF], F32)
                    kh = psh.tile([P, SBF], F32)
                    if "mm" not in ablate:
                        # Contiguous accumulation group per 512-col PSUM
                        # slice; interleaving two open groups stalls the PE.
                        for j in range(SF):
                            for di in range(DT):
                                nc.tensor.matmul(
                                    kz[:, j * SB:(j + 1) * SB],
                                    wz_sb[:, di, hi * P:(hi + 1) * P],
                                    x_t[j][:, di, :],
                                    start=(di == 0),
                                    stop=(di == DT - 1),
                                )
                        for j in range(SF):
                            for di in range(DT):
                                nc.tensor.matmul(
                                    kh[:, j * SB:(j + 1) * SB],
                                    wh_sb[:, di, hi * P:(hi + 1) * P],
                                    x_t[j][:, di, :],
                                    start=(di == 0),
                                    stop=(di == DT - 1),
                                )

                    bcol = (hi, hi + 1)
                    zt = inter.tile([P, SBF], F32, tag="z")
                    ct = inter.tile([P, SBF], F32, tag="c")
                    at = inter.tile([P, SBF], F32, tag="a")
                    tl = inter.tile([P, SBF], F32, tag="tl")
                    vt = inter.tile([P, SBF], F32, tag="v")
                    if scheme == "A":
                        if "act" not in ablate:
                            nc.scalar.activation(
                                zt[:], kz[:], AF.Sigmoid,
                                bias=bz_sb[:, bcol[0]:bcol[1]],
                            )
                            nc.scalar.activation(
                                ct[:], kz[:], AF.Sigmoid,
                                bias=nbz_sb[:, bcol[0]:bcol[1]], scale=-1.0,
                            )
                            nc.scalar.activation(
                                at[:], kh[:], AF.Sigmoid,
                                bias=bh_sb[:, bcol[0]:bcol[1]],
                            )
                        if "dve" not in ablate:
                            # tilde = g(kh+bh) = max(kh+bh+0.5, sig(kh+bh))
                            nc.vector.scalar_tensor_tensor(
                                tl[:], kh[:], bhh_sb[:, bcol[0]:bcol[1]], at[:],
                                op0=OP.add, op1=OP.max,
                            )
                            vt_eng.tensor_mul(vt[:], zt[:], tl[:])
                    else:  # scheme E
                        mt = inter.tile([P, SBF], F32, tag="m")
                        if "act" not in ablate:
                            nc.scalar.activation(
                                zt[:], kz[:], AF.Sigmoid,
                                bias=bz_sb[:, bcol[0]:bcol[1]],
                            )
                            nc.scalar.activation(
                                at[:], kh[:], AF.Sigmoid,
                                bias=bh_sb[:, bcol[0]:bcol[1]],
                            )
                            nc.scalar.activation(
                                mt[:], kh[:], AF.Identity,
                                bias=bhh_sb[:, bcol[0]:bcol[1]],
                            )
                        if "dve" not in ablate:
                            nc.vector.tensor_scalar(
                                ct[:], zt[:], -1.0, 1.0,
                                op0=OP.mult, op1=OP.add,
                            )
                            nc.vector.tensor_max(tl[:], at[:], mt[:])
                            vt_eng.tensor_mul(vt[:], zt[:], tl[:])

                    ot = outp.tile([P, SBF], F32, tag="o")
                    if "scan" not in ablate:
                        init = (
                            g0[:, hi:hi + 1] if sbp == 0
                            else prev_out[hi][:, SBF - 1:SBF]
                        )
                        nc.vector.tensor_tensor_scan(
                            ot[:], ct[:], vt[:], init, op0=OP.mult, op1=OP.add
                        )
                        prev_out[hi] = ot
                    if "outdma" not in ablate:
                        nc.sync.dma_start(
                            out=hT_v[:, hi, sbp * SBF:(sbp + 1) * SBF], in_=ot[:]
                        )
    nc.finalize()
    return nc


def _get_program():
    if "nc" not in _CACHE:
        _CACHE["nc"] = _build_program()
    return _CACHE["nc"]


def run(inputs, **kw):
    """Run on hardware; returns (output (B,S,H) fp32, BassKernelResults)."""
    x = np.asarray(inputs["x"], dtype=np.float32)
    h0 = np.asarray(inputs["h0"], dtype=np.float32)
    Wz = np.asarray(inputs["Wz"], dtype=np.float32)
    bz = np.asarray(inputs["bz"], dtype=np.float32)
    Wh = np.asarray(inputs["Wh"], dtype=np.float32)
    bh = np.asarray(inputs["bh"], dtype=np.float32)

    mm_np = mybir.dt.np(MM_DT)
    wzT = np.ascontiguousarray(Wz.T).astype(mm_np)
    whT = np.ascontiguousarray(Wh.T).astype(mm_np)
    bzg = np.ascontiguousarray(bz.reshape(HT, P).T)
    bhg = np.ascontiguousarray(bh.reshape(HT, P).T)

    in_maps = []
    for b in range(N_CORES):
        in_maps.append({
            "xt": np.ascontiguousarray(x[b].T).astype(mm_np),
            "wzt": wzT,
            "wht": whT,
            "bzg": bzg,
            "bhg": bhg,
            "h0g": np.ascontiguousarray(h0[b, 0].reshape(HT, P).T),
        })

    nc = _get_program()
    res = run_bass_kernel_spmd(nc, in_maps, core_ids=list(range(N_CORES)), **kw)
    out = np.stack([res.results[b]["ht"].T for b in range(N_CORES)], axis=0)
    return np.ascontiguousarray(out), res


def kernel(**inputs):
    out, _ = run(inputs)
    return out
